# revision 15
# baseline (speedup 1.0000x reference)
"""Trainium2 Bass kernel for a ResNet BasicBlock (dense CNN, sync-BN).

Reference computation (training-mode BN, batch stats over (N,H,W)):
    h = conv3x3(x, W1) * mask1            # structured channel pruning
    h = relu(bn(h, gamma1, beta1))
    h = conv3x3(h, W2) * mask2
    h = bn(h, gamma2, beta2)
    out = relu(h + x)                      # identity shortcut

Shapes: x [32, 256, 56, 56] f32, W [256, 256, 3, 3] f32.

Strategy: data-parallel over batch N across 8 NeuronCores (4 images per
core), weights replicated.  BN batch statistics are synchronized with a
tiny AllReduce of per-channel (sum, sum-of-squares) pairs whose DMA
descriptors are pre-generated at kernel start and fired by a single
gpsimd trigger when the stats are ready.

Channel sparsity: mask1 zeroes ~half of conv1's output channels, and
(when beta1 <= 0 on those channels, which holds for the reference
inputs) the corresponding h1 channels are exactly zero, so conv1 only
computes the a1=|mask1| active channels and conv2 only consumes them.
Active channels are packed into partition groups of <=128 via a
host-side permutation folded into the weights:
  - conv1 output groups: [128, a1-128]; the small overflow group's
    conv2 contribution is evaluated as ONE im2col matmul per chunk
    (K = 9 taps * (a1-128) channels <= 128) against a plane holding 9
    tap-shifted replicas of the overflow channels (built by SBUF->SBUF
    DMAs; the center tap block sits at partition 0 so compute APs stay
    partition-aligned, and the block order is folded into the weights).
  - conv2 output stays in TRUE channel layout (inactive mask2 rows are
    zeroed in the weights), so the residual tail needs no scatter.

BN statistics are produced by the DVE bn_stats/bn_aggr instructions
(one Welford 6-tuple per PSUM chunk), so the ACT engine only does the
PSUM->SBUF drain copies; ACT traffic is what contends with the PE's
SBUF rhs stream, so keeping it minimal preserves matmul pitch.

Per-core layout: zero-padded 58x58 bf16 planes so each conv tap is an
offset shift; 7 chunks of 8 output rows per image so a chunk fits a
PSUM bank.  Head loads / BN1 applies are emitted interleaved with the
conv work (one image of lookahead) so the strict-FIFO ACT/DVE queues
never dam up behind bulk phases.
"""

import numpy as np
import ml_dtypes

# ---- problem constants (hardcoded; kernel.py must be self-contained) ----
N_TOT, C, H, W = 32, 256, 56, 56
N_CORES = 8
NL = N_TOT // N_CORES          # images per core
PW = H + 2                     # padded row stride (58)
PLANE = PW * PW + 4            # padded plane floats + 4 spare for tap overreads
STRIP0 = PW + 1                # first interior output position (59)
CHUNK = 8 * PW                 # 464: 8 output rows per chunk
NCHUNK = 7                     # 7 chunks * 8 rows = 56 rows
HW = H * W                     # 3136
HALF_ROWS = 28                 # row granularity for x/out streaming DMAs
HALF_ELEMS = HALF_ROWS * W     # 1568
COUNT = N_TOT * HW             # sync-BN element count per channel
CL = NL * HW                   # local per-core element count per channel
EPS = 1e-5

_BF16 = ml_dtypes.bfloat16

_cache = {}

TAPS = [(ky, kx) for ky in range(3) for kx in range(3)]
BORD = [4, 0, 1, 2, 3, 5, 6, 7, 8]   # i2c plane tap-block order, center first


def _make_plan(mask1, beta1):
    """Conv1 output channel groups (== conv2 input groups), true-channel ids."""
    act1 = np.flatnonzero(mask1 != 0)
    inact = np.flatnonzero(mask1 == 0)
    if np.any(np.maximum(beta1[inact], 0.0) != 0.0) or len(act1) == 0:
        act1 = np.arange(C)    # dense fallback: every channel treated live
    groups = [act1[i:i + 128] for i in range(0, len(act1), 128)]
    kinds = ["i2c" if (9 * len(g) <= 128 and len(g) < 128) else "full"
             for g in groups]
    return groups, kinds


def _pack_weights(W1, W2, mask2, groups, kinds):
    m2 = mask2.reshape(2, 128).astype(np.float32)
    packs = {}
    for gi, (g, kind) in enumerate(zip(groups, kinds)):
        s = len(g)
        blocks = []
        for h in range(2):
            for (ky, kx) in TAPS:
                blocks.append(W1[g, 128 * h:128 * h + 128, ky, kx].T)  # [ci,co]
        packs[f"wt1_{gi}"] = np.ascontiguousarray(
            np.concatenate(blocks, axis=1)).astype(_BF16)      # [128, 18*s]
        if kind == "full":
            blocks = []
            for j in range(2):
                for (ky, kx) in TAPS:
                    blk = (W2[128 * j:128 * j + 128, :, ky, kx][:, g]
                           * m2[j][:, None]).T                  # [s, 128co]
                    blocks.append(blk)
            packs[f"wt2m_{gi}"] = np.ascontiguousarray(
                np.concatenate(blocks, axis=1)).astype(_BF16)   # [s, 18*128]
        else:
            # block order: center tap first so the plane's compute-written
            # rows sit at partition base 0 (engine AP alignment rule)
            arr = np.zeros((9 * s, 256), np.float32)
            for b, t in enumerate(BORD):
                ky, kx = TAPS[t]
                for j in range(2):
                    arr[b * s:(b + 1) * s, j * 128:(j + 1) * 128] = \
                        (W2[128 * j:128 * j + 128, :, ky, kx][:, g]
                         * m2[j][:, None]).T
            packs[f"wt2o_{gi}"] = arr.astype(_BF16)             # [9s, 256]
    return packs


def _pack_aff(gamma1, beta1, gamma2, beta2, groups):
    G = len(groups)
    out = np.zeros((128, 2 * G + 4), np.float32)
    for gi, g in enumerate(groups):
        s = len(g)
        out[:s, gi] = gamma1[g]
        out[:s, G + gi] = beta1[g]
    g2 = np.asarray(gamma2, np.float32).reshape(2, 128)
    b2 = np.asarray(beta2, np.float32).reshape(2, 128)
    out[:, 2 * G + 0] = g2[0]
    out[:, 2 * G + 1] = g2[1]
    out[:, 2 * G + 2] = b2[0]
    out[:, 2 * G + 3] = b2[1]
    return out


def _build(groups, kinds):
    import concourse.bass as bass_mod
    import concourse.bacc as bacc
    import concourse.mybir as mybir
    import concourse.tile as tile

    f32 = mybir.dt.float32
    bf16 = mybir.dt.bfloat16
    AX = mybir.AxisListType
    ALU = mybir.AluOpType
    AF = mybir.ActivationFunctionType

    G = len(groups)
    sizes = [len(g) for g in groups]
    SW = 2 * G                       # stats-exchange width (sums | sumsqs)

    nc = bacc.Bacc("TRN2", target_bir_lowering=False, debug=False,
                   num_devices=N_CORES)

    x_d = nc.dram_tensor("x", [NL, C, H, W], f32, kind="ExternalInput")
    wt1_d = [nc.dram_tensor(f"wt1_{gi}", [128, 18 * sizes[gi]], bf16,
                            kind="ExternalInput") for gi in range(G)]
    wt2_d = []
    for gi in range(G):
        if kinds[gi] == "full":
            wt2_d.append(nc.dram_tensor(f"wt2m_{gi}", [sizes[gi], 18 * 128],
                                        bf16, kind="ExternalInput"))
        else:
            wt2_d.append(nc.dram_tensor(f"wt2o_{gi}", [9 * sizes[gi], 256],
                                        bf16, kind="ExternalInput"))
    aff_d = nc.dram_tensor("aff", [128, SW + 4], f32, kind="ExternalInput")
    out_d = nc.dram_tensor("out", [NL, C, H, W], f32, kind="ExternalOutput")

    replica_groups = [list(range(N_CORES))]

    def interior(tile_ap, base, nrows):
        """[p, nrows, 56] strided view (row stride PW) starting at `base`."""
        v = tile_ap[:, base:base + nrows * PW].rearrange(
            "p (r c) -> p r c", c=PW)
        return v[:, :, 0:W]

    with tile.TileContext(nc) as tc:
        import contextlib
        with contextlib.ExitStack() as ctx:
            const = ctx.enter_context(tc.tile_pool(name="const", bufs=1))
            psum = ctx.enter_context(tc.tile_pool(name="psum", bufs=6,
                                                  space="PSUM"))
            psumb = ctx.enter_context(tc.tile_pool(name="psumb", bufs=2,
                                                   space="PSUM"))
            xst = ctx.enter_context(tc.tile_pool(name="xst", bufs=3))
            otp = ctx.enter_context(tc.tile_pool(name="otp", bufs=2))
            sqp = ctx.enter_context(tc.tile_pool(name="sqp", bufs=2))

            wt1_sb = [const.tile([128, 18 * sizes[gi]], bf16,
                                 tag=f"wt1_{gi}", name=f"wt1_{gi}")
                      for gi in range(G)]
            wt2_sb = []
            for gi in range(G):
                if kinds[gi] == "full":
                    wt2_sb.append(const.tile([sizes[gi], 18 * 128], bf16,
                                             tag=f"wt2_{gi}", name=f"wt2_{gi}"))
                else:
                    wt2_sb.append(const.tile([9 * sizes[gi], 256], bf16,
                                             tag=f"wt2_{gi}", name=f"wt2_{gi}"))
            for gi in range(G):
                nc.sync.dma_start(wt1_sb[gi][:], wt1_d[gi][:])
                nc.sync.dma_start(wt2_sb[gi][:], wt2_d[gi][:])
            aff_sb = const.tile([128, SW + 4], f32, tag="aff", name="aff")
            nc.sync.dma_start(aff_sb[:], aff_d[:])

            # ---- cross-core stats exchange plumbing (SBUF remote DMA) ----
            # Two exchanges (BN1, BN2).  Each broadcasts this core's packed
            # [128, SW(=4)] stats tile to all 7 peers with XOR-relative
            # dests; slot d of the receive tile gets the copy from core
            # (me ^ d).  Hardware remote sems count arrivals (2 per
            # transfer -> wait >= 14).  Descriptors are PRE-GENERATED here
            # (they only record addresses); a single gpsimd trigger_dma
            # fires each batch of 7 once the stats tile is written.
            rsem = [nc.alloc_semaphore(f"rst{i}") for i in range(2)]
            lsem = nc.alloc_semaphore("lst")
            _gp_prev = [None]
            deferred_waits = []

            def gp_order(bi):
                if _gp_prev[0] is not None:
                    bass_mod._add_dep_helper(bi.ins, _gp_prev[0].ins,
                                             sync=False,
                                             reason="stats-exchange order")
                _gp_prev[0] = bi
                return bi

            nc._bir_kernel_barrier_sem_replica_groups.extend(
                set(g) for g in replica_groups)

            def defer_wait(bi, sem, val):
                bi._wait_ge(sem, 0)
                deferred_waits.append((bi, sem, val))
                return bi

            # sems persist across NEFF executions: clear them as soon as all
            # cores have entered (peers send >100us later, after conv1)
            for i, s in enumerate(rsem + [lsem]):
                cl = gp_order(nc.gpsimd.sem_clear(s))
                if i == 0:
                    defer_wait(cl, nc._bir_kernel_barrier_sem,
                               nc.bir_kernel_barrier_sem_inc)

            # stats tiles written at conv end; descriptors reference them now
            packed1 = const.tile([128, SW], f32, tag="pk1", name="pk1")
            packed2 = const.tile([128, 4], f32, tag="pk2", name="pk2")
            rv1 = const.tile([128, 8 * SW], f32, tag="rv1", name="rv1")
            rv2 = const.tile([128, 32], f32, tag="rv2", name="rv2")
            for ex, (pk, rv, w) in enumerate(((packed1, rv1, SW),
                                              (packed2, rv2, 4))):
                for d in range(1, 8):
                    rd = [None] * 8
                    rd[d] = (0, d)
                    gp_order(nc.gpsimd.remote_dma_broadcast(
                        rv[:, w * d:w * d + w], pk[:],
                        remote_sem=rsem[ex], local_sem=lsem, rdests=rd))

            # persistent per-image planes
            x_pad = [[const.tile([128, PLANE], bf16, tag=f"xp{j}_{n}",
                                 name=f"xp{j}_{n}")
                      for n in range(NL)] for j in range(2)]
            h1 = []                       # per group: list over images
            for gi in range(G):
                rows = 9 * sizes[gi] if kinds[gi] == "i2c" else sizes[gi]
                h1.append([const.tile([rows, PLANE], bf16, tag=f"h1{gi}_{n}",
                                      name=f"h1{gi}_{n}")
                           for n in range(NL)])
            h2 = [[const.tile([128, HW], bf16, tag=f"h2{j}_{n}",
                              name=f"h2{j}_{n}")
                   for n in range(NL)] for j in range(2)]

            # zero the non-interior positions of padded planes (i2c planes:
            # center block only; other blocks are fully DMA-overwritten with
            # shifted copies whose source pads are these zeros)
            def zero_pads(t, s):
                tt = t[0:s]
                nc.vector.memset(tt[:, 0:STRIP0], 0.0)
                pairs = tt[:, 2 * PW - 1:2 * PW - 1 + 56 * PW].rearrange(
                    "p (r c) -> p r c", c=PW)[:, :, 0:2]
                nc.vector.memset(pairs, 0.0)
                nc.vector.memset(tt[:, STRIP0 + 56 * PW:PLANE], 0.0)

            for j in range(2):
                for n in range(NL):
                    zero_pads(x_pad[j][n], 128)
            for gi in range(G):
                for n in range(NL):
                    zero_pads(h1[gi][n], sizes[gi])

            # per-(image,chunk) (sum, sumsq) accumulator columns, filled by
            # the ACT drain copies' accum_out and a paired ACT square op
            acc1 = {(gi, sq): const.tile([sizes[gi], NL * NCHUNK], f32,
                                         tag=f"a1{gi}{sq}", name=f"a1{gi}{sq}")
                    for gi in range(G) for sq in ("s", "q")}
            acc2 = {(j, sq): const.tile([128, NL * NCHUNK], f32,
                                        tag=f"a2{j}{sq}", name=f"a2{j}{sq}")
                    for j in range(2) for sq in ("s", "q")}

            # ---- head: stream x in (all DMAs up front; staging ring 4),
            # casts emitted per image inside the conv1 loop below ----
            head_xs = []
            for n in range(NL):
                for rh in range(2):
                    for j in range(2):
                        r0 = rh * HALF_ROWS
                        xs = xst.tile([128, HALF_ELEMS], f32, tag="xs",
                                      name="xs")
                        nc.sync.dma_start(
                            xs[:],
                            x_d[n, j * 128:(j + 1) * 128, r0:r0 + HALF_ROWS, :])
                        head_xs.append((n, rh, j, xs))

            def emit_casts(n):
                for (nn, rh, j, xs) in head_xs:
                    if nn != n:
                        continue
                    r0 = rh * HALF_ROWS
                    dst = interior(x_pad[j][nn], (r0 + 1) * PW + 1, HALF_ROWS)
                    src = xs[:, :].rearrange("p (r c) -> p r c", c=W)
                    nc.vector.tensor_copy(dst, src)

            # ---- conv1: per chunk, one 18-matmul run per output group
            # (runs kept contiguous per PSUM group so the PE's LDWEIGHTS
            # pull-ahead pipelining stays active) ----
            emit_casts(0)
            emit_casts(1)
            c1_last = None
            for n in range(NL):
                if n + 2 < NL:
                    emit_casts(n + 2)
                for k in range(NCHUNK):
                    col = n * NCHUNK + k
                    for gi in range(G):
                        s = sizes[gi]
                        pool = psum if kinds[gi] == "full" else psumb
                        tag = "ps" if kinds[gi] == "full" else "psb"
                        pt = pool.tile([s, 8 * W], f32, tag=tag, name=tag)
                        for idx, (hh, (ky, kx)) in enumerate(
                                (hh, t) for hh in range(2) for t in TAPS):
                            dq = (ky - 1) * PW + (kx - 1)
                            off = STRIP0 + CHUNK * k + dq
                            rhs = x_pad[hh][n][:, off:off + CHUNK].rearrange(
                                "p (r c) -> p r c", c=PW)[:, :, 0:W]
                            nc.tensor.matmul(
                                pt[:], wt1_sb[gi][:, idx * s:(idx + 1) * s],
                                rhs, start=(idx == 0), stop=(idx == 17))
                        base = (1 + 8 * k) * PW + 1
                        dst = interior(h1[gi][n][0:s], base, 8)
                        src = pt[:, 0:8 * W].rearrange("p (r c) -> p r c", c=W)
                        nc.scalar.activation(
                            dst, src, AF.Copy,
                            accum_out=acc1[(gi, "s")][:, col:col + 1])
                        sq = sqp.tile([128, 8 * W], f32, tag="sq", name="sq")
                        c1_last = nc.scalar.activation(
                            sq[0:s, :].rearrange("p (r c) -> p r c", c=W),
                            dst, AF.Square,
                            accum_out=acc1[(gi, "q")][:, col:col + 1])

            # ---- BN1 stats: aggregate -> (sum, sumsq) -> fire trigger ----
            def emit_stats(accs, accq, packed, w_i, sq_i, s):
                nc.vector.tensor_reduce(
                    packed[0:s, w_i:w_i + 1], accs[:], axis=AX.X, op=ALU.add)
                return nc.vector.tensor_reduce(
                    packed[0:s, sq_i:sq_i + 1], accq[:], axis=AX.X, op=ALU.add)

            for gi in range(G):
                red1 = emit_stats(acc1[(gi, "s")], acc1[(gi, "q")], packed1,
                                  gi, G + gi, sizes[gi])
            nc.vector.tensor_copy(rv1[:, 0:SW], packed1[:])   # own slot (d=0)
            tr1 = gp_order(nc.gpsimd.trigger_dma(count=7))
            bass_mod._add_dep_helper(tr1.ins, red1.ins, sync=True,
                                     reason="stats1 ready")

            # ---- wait for all 8 contributions, sum slots -> global stats ----
            gl1 = const.tile([128, SW], f32, tag="gl1", name="gl1")
            rec1 = nc.vector.tensor_reduce(
                gl1[:], rv1[:, 0:8 * SW].rearrange("p (s c) -> p c s", c=SW),
                axis=AX.X, op=ALU.add)
            defer_wait(rec1, rsem[0], 14)
            bass_mod._add_dep_helper(rec1.ins, c1_last.ins, sync=True,
                                     reason="recv after conv phase")

            # ---- BN affine from global stats (DVE-only; fast rsqrt) ----
            def bn_affine(gl, w, g_col, b_col, sfx):
                """gl [128, 2w] = sums | sumsqs -> (scale, bias) [128, w]."""
                mean = const.tile([128, w], f32, tag=f"mean{sfx}",
                                  name=f"mean{sfx}")
                nc.vector.tensor_scalar_mul(mean[:], gl[:, 0:w], 1.0 / COUNT)
                var = const.tile([128, w], f32, tag=f"var{sfx}",
                                 name=f"var{sfx}")
                nc.vector.tensor_tensor(var[:], mean[:], mean[:], ALU.mult)
                nc.vector.scalar_tensor_tensor(
                    var[:], gl[:, w:2 * w], 1.0 / COUNT, var[:],
                    ALU.mult, ALU.subtract)
                nc.vector.tensor_scalar_add(var[:], var[:], EPS)
                y = const.tile([128, w], f32, tag=f"y{sfx}", name=f"y{sfx}")
                vh = const.tile([128, w], f32, tag=f"vh{sfx}", name=f"vh{sfx}")
                tmp = const.tile([128, w], f32, tag=f"tm{sfx}",
                                 name=f"tm{sfx}")
                iv = var[:].bitcast(mybir.dt.int32)
                yi = y[:].bitcast(mybir.dt.int32)
                nc.vector.tensor_scalar(yi, iv, 1, None, ALU.arith_shift_right)
                nc.vector.tensor_scalar(yi, yi, -1, None, ALU.bitwise_xor)
                nc.vector.tensor_scalar(yi, yi, 0x5f3759df + 1, None, ALU.add)
                nc.vector.tensor_scalar_mul(vh[:], var[:], 0.5)
                for _ in range(2):
                    nc.vector.tensor_tensor(tmp[:], y[:], y[:], ALU.mult)
                    nc.vector.tensor_tensor(tmp[:], tmp[:], vh[:], ALU.mult)
                    nc.vector.tensor_scalar(tmp[:], tmp[:], -1.0, 1.5,
                                            ALU.mult, ALU.add)
                    nc.vector.tensor_tensor(y[:], y[:], tmp[:], ALU.mult)
                sc = const.tile([128, w], f32, tag=f"sc{sfx}", name=f"sc{sfx}")
                nc.vector.tensor_tensor(sc[:], aff_sb[:, g_col:g_col + w],
                                        y[:], ALU.mult)
                bi = const.tile([128, w], f32, tag=f"bi{sfx}", name=f"bi{sfx}")
                nc.vector.tensor_tensor(bi[:], mean[:], sc[:], ALU.mult)
                nc.vector.tensor_tensor(bi[:], aff_sb[:, b_col:b_col + w],
                                        bi[:], ALU.subtract)
                return sc, bi

            sc1, bi1 = bn_affine(gl1, G, 0, G, "1")

            # ---- per image: BN1 apply (ACT relu, rh-major so conv2's first
            # chunks unblock after two ops) + tap-shifted replication of the
            # applied i2c center block; then conv2 for that image ----
            def emit_apply(n):
                for rh in range(2):
                    base = (1 + rh * HALF_ROWS) * PW + 1
                    for gi in range(G):
                        s = sizes[gi]
                        v = interior(h1[gi][n][0:s], base, HALF_ROWS)
                        nc.scalar.activation(
                            v, v, AF.Relu,
                            bias=bi1[0:s, gi:gi + 1],
                            scale=sc1[0:s, gi:gi + 1])
                for gi in range(G):
                    if kinds[gi] != "i2c":
                        continue
                    s = sizes[gi]
                    for b, t in enumerate(BORD):
                        if b == 0:
                            continue
                        ky, kx = TAPS[t]
                        dq = (ky - 1) * PW + (kx - 1)
                        nc.sync.dma_start(
                            h1[gi][n][b * s:(b + 1) * s,
                                      STRIP0:STRIP0 + 56 * PW],
                            h1[gi][n][0:s,
                                      STRIP0 + dq:STRIP0 + 56 * PW + dq])

            c2_last = None
            emit_apply(0)
            for n in range(NL):
                if n + 1 < NL:
                    emit_apply(n + 1)
                for k in range(NCHUNK):
                    col = n * NCHUNK + k
                    for j in range(2):
                        pt = psum.tile([128, 8 * W], f32, tag="ps", name="ps")
                        nmm = sum(9 if kinds[gi] == "full" else 1
                                  for gi in range(G))
                        idx = 0
                        for gi in range(G):
                            s = sizes[gi]
                            if kinds[gi] == "full":
                                for t, (ky, kx) in enumerate(TAPS):
                                    dq = (ky - 1) * PW + (kx - 1)
                                    off = STRIP0 + CHUNK * k + dq
                                    rhs = h1[gi][n][0:s, off:off + CHUNK] \
                                        .rearrange("p (r c) -> p r c",
                                                   c=PW)[:, :, 0:W]
                                    nc.tensor.matmul(
                                        pt[:],
                                        wt2_sb[gi][:, (j * 9 + t) * 128:
                                                   (j * 9 + t + 1) * 128],
                                        rhs, start=(idx == 0),
                                        stop=(idx == nmm - 1))
                                    idx += 1
                            else:
                                off = STRIP0 + CHUNK * k
                                rhs = h1[gi][n][0:9 * s, off:off + CHUNK] \
                                    .rearrange("p (r c) -> p r c",
                                               c=PW)[:, :, 0:W]
                                nc.tensor.matmul(
                                    pt[:],
                                    wt2_sb[gi][:, j * 128:(j + 1) * 128],
                                    rhs, start=(idx == 0),
                                    stop=(idx == nmm - 1))
                                idx += 1
                        dst = h2[j][n][:, 8 * k * W:(8 * k + 8) * W] \
                            .rearrange("p (r c) -> p r c", c=W)
                        src = pt[:, 0:8 * W].rearrange("p (r c) -> p r c", c=W)
                        nc.scalar.activation(
                            dst, src, AF.Copy,
                            accum_out=acc2[(j, "s")][:, col:col + 1])
                        sq = sqp.tile([128, 8 * W], f32, tag="sq", name="sq")
                        c2_last = nc.scalar.activation(
                            sq[:, :].rearrange("p (r c) -> p r c", c=W),
                            dst, AF.Square,
                            accum_out=acc2[(j, "q")][:, col:col + 1])

            # ---- BN2 stats -> exchange -> affine ----
            for j in range(2):
                red2 = emit_stats(acc2[(j, "s")], acc2[(j, "q")], packed2,
                                  j, 2 + j, 128)
            nc.vector.tensor_copy(rv2[:, 0:4], packed2[:])
            tr2 = gp_order(nc.gpsimd.trigger_dma(count=7))
            bass_mod._add_dep_helper(tr2.ins, red2.ins, sync=True,
                                     reason="stats2 ready")

            gl2 = const.tile([128, 4], f32, tag="gl2", name="gl2")
            rec2 = nc.vector.tensor_reduce(
                gl2[:], rv2[:, 0:32].rearrange("p (s c) -> p c s", c=4),
                axis=AX.X, op=ALU.add)
            defer_wait(rec2, rsem[1], 14)
            bass_mod._add_dep_helper(rec2.ins, c2_last.ins, sync=True,
                                     reason="recv after conv phase")
            sc2, bi2 = bn_affine(gl2, 2, SW, SW + 2, "2")

            # ---- tail: out = relu(sc2*h2 + bi2 + x), stream to DRAM.
            # The 32 elementwise ops are spread over DVE/GPSIMD/ACT so no
            # single engine's serial chain gates the (DMA-bound) tail. ----
            for p, (n, rh, j) in enumerate((n, rh, j) for n in range(NL)
                                           for rh in range(2)
                                           for j in range(2)):
                r0 = rh * HALF_ROWS
                xv = interior(x_pad[j][n], (r0 + 1) * PW + 1, HALF_ROWS)
                h2v = h2[j][n][:, r0 * W:r0 * W + HALF_ELEMS].rearrange(
                    "p (r c) -> p r c", c=W)
                tb = otp.tile([128, HALF_ELEMS], bf16, tag="tb",
                              name="tb", bufs=2)
                tbv = tb[:, :].rearrange("p (r c) -> p r c", c=W)
                nc.vector.scalar_tensor_tensor(
                    tbv, h2v, sc2[:, j:j + 1], xv, ALU.mult, ALU.add)
                pool = otp if p % 2 == 0 else xst
                ot = pool.tile([128, HALF_ELEMS], f32,
                               tag="ot" if p % 2 == 0 else "xs",
                               name="ot")
                nc.scalar.activation(ot[:], tb[:], AF.Relu,
                                     bias=bi2[:, j:j + 1], scale=1.0)
                nc.sync.dma_start(
                    out_d[n, j * 128:(j + 1) * 128, r0:r0 + HALF_ROWS, :],
                    ot[:])

    # patch the reserved wait slots to their real thresholds now that
    # scheduling is done (the single-core scheduling simulator cannot
    # satisfy remote increments)
    for bi, sem, val in deferred_waits:
        patched = False
        for wv in bi.ins.sync_info.on_wait:
            if wv.id == sem.num and wv.wait_value == 0:
                wv.wait_value = val
                patched = True
                break
        assert patched, f"deferred wait not found on {bi.ins.name}"

    nc.compile()
    return nc


def kernel(x, W1, W2, gamma1, beta1, gamma2, beta2, mask1, mask2,
           _trace=False, _trace_kwargs=None):
    from concourse.bass_utils import run_bass_kernel_spmd

    mask1 = np.asarray(mask1, np.float32)
    mask2 = np.asarray(mask2, np.float32)
    beta1 = np.asarray(beta1, np.float32)
    groups, kinds = _make_plan(mask1, beta1)
    key = (tuple(len(g) for g in groups), tuple(kinds))
    if _cache.get("key") != key:
        _cache["nc"] = _build(groups, kinds)
        _cache["key"] = key
    nc = _cache["nc"]

    packs = _pack_weights(np.asarray(W1, np.float32),
                          np.asarray(W2, np.float32), mask2, groups, kinds)
    aff = _pack_aff(np.asarray(gamma1, np.float32), beta1,
                    np.asarray(gamma2, np.float32),
                    np.asarray(beta2, np.float32), groups)
    x = np.ascontiguousarray(np.asarray(x, np.float32))

    in_maps = [dict(packs, x=x[i * NL:(i + 1) * NL], aff=aff)
               for i in range(N_CORES)]
    kw = {}
    if _trace:
        kw = dict(trace=True, **(_trace_kwargs or {}))
    res = run_bass_kernel_spmd(nc, in_maps, core_ids=list(range(N_CORES)), **kw)
    out = np.concatenate([res.results[i]["out"] for i in range(N_CORES)],
                         axis=0)
    _cache["last_results"] = res
    return out
